# revision 1
# baseline (speedup 1.0000x reference)
"""Trainium2 Bass kernel for nn_BehavioralCircuit — pipelined 3-stage solver.

Reference: T=100000 sequential steps of a reward-modulated Hebbian rule over
512 independent 2-D units:
    r[t] = rewards[t] - movavg10(rewards)[t];  u = LR*r
    h    = sigmoid(W @ x_t);  m[t] = h.mean();  W += u[t] * outer(h, x_t)

Scheme (validated in sim + against an exact numpy mirror of the device
arithmetic; full-run rel err ~7.6e-4):
  Blocks of TAU=126 steps.  Within block b (64 units per core):
    seed : h0 = sigmoid(X_b.W(b-1))      [2-stale W + mtil2@h1(b-2) base]
    mid  : h1 = sigmoid(X_b.W(b-1) + mtil@h1(b-1) + K@h0)
    final: h2 = sigmoid(X_b.W(b) + K@h1) [exact base via mtil@(h2-h1)(b-1)]
    W(b) = W(b-1) + c(b-1)^T h2(b-1),  c = u*x
  K[t,s] = u_s (x_t.x_s) [s<t]; mtil/mtil2 couple adjacent blocks.  All
  per-block lhsT tiles are precomputed on the host and DMA-streamed.

Device per block: 7 matmuls (mtil2@h1(b-2), X@W stale (K=2), (c|mtil)@h1(b-1),
K@h0, K@(-h0), K@h1, (c|mtil)@d21(b-1)), 3 sigmoids on ACT, DVE: d21/neg/
W-updates.  h2 tiles are DMA'd out; the host does the final unit-mean.
The only cross-block serial chain is sigma2 -> d21 -> comb2d -> sigma2; the
W bookkeeping and the seed/mid stages are pipelined 1-2 blocks ahead.

All tiles are full 128-partition, base 0 (HW restriction on partition
offsets): rows 0:2 carry dW = c^T@h (or junk on h tiles, killed by zero
rows/cols in the streamed lhsT tiles), rows 2:128 are the 126 steps.
"""

import sys

import numpy as np

sys.path.insert(0, "/opt/trn_rl_repo")

import concourse.bass as bass
import concourse.bacc as bacc
import concourse.tile as tile
from concourse import mybir
from concourse.bass_utils import run_bass_kernel_spmd

TAU = 126
MP = 128            # tile partition size = 2 (dW rows) + TAU
T_FULL = 100000
NB_FULL = (T_FULL + TAU - 1) // TAU   # 794
NH = 512
NCORES = 8
UH = NH // NCORES   # 64
CH = 32             # stream chunk, blocks per DMA
LR = 0.1
WINDOW = 10

F32 = mybir.dt.float32
F16 = mybir.dt.float16
AF = mybir.ActivationFunctionType
OP = mybir.AluOpType


# ---------------------------------------------------------------------------
# Host-side stream preparation (shared across cores)
# ---------------------------------------------------------------------------

def _movavg_u(rewards, t_pad):
    cs = np.cumsum(rewards, dtype=np.float64)
    sh = np.concatenate([np.zeros(WINDOW), cs[:-WINDOW]])
    wsum = cs - sh
    count = np.minimum(np.arange(len(rewards)) + 1.0, float(WINDOW))
    u = (LR * (rewards - wsum / count)).astype(np.float32)
    up = np.zeros((t_pad,), np.float32)
    up[: len(rewards)] = u
    return up


def prep_streams(X, rewards, nb):
    t_pad = nb * TAU
    Xp = np.zeros((t_pad, 2), np.float32)
    Xp[: X.shape[0]] = X
    up = _movavg_u(rewards, t_pad)
    Xb = Xp.reshape(nb, TAU, 2)              # [b, t, 2]
    ub = up.reshape(nb, TAU)                 # [b, t]

    # kT[b][2+s, 2+t] = u_b[s] * (x_bs . x_bt) * [s < t]; rows/cols 0:2 = 0
    G = np.einsum("btc,bsc->bst", Xb, Xb)            # [b, s, t]
    kTc = G * ub[:, :, None]                         # u_s * (xs.xt)
    smask = np.triu(np.ones((TAU, TAU), np.float32), 1)  # [s,t]: s<t
    kT = np.zeros((nb, MP, MP), np.float32)
    kT[:, 2:, 2:] = kTc * smask[None]
    kT = kT.astype(np.float16)

    # CM[b] (b>=1): lhsT, contraction over rows 2+t' (h1(b-1) rows):
    #   cols 0:2  = c(b-1)[t', :] = u_{b-1}[t'] * x_{b-1, t'}
    #   cols 2+t  = mtilT[t', t]  = u_{b-1}[t'] * (x_{b-1,t'} . x_{b,t})
    CM = np.zeros((nb, MP, MP), np.float32)
    CM[1:, 2:, 0:2] = ub[:-1, :, None] * Xb[:-1]
    cross1 = np.einsum("btc,bsc->bst", Xb[1:], Xb[:-1])   # [j, t', t]
    CM[1:, 2:, 2:] = cross1 * ub[:-1, :, None]
    CM = CM.astype(np.float16)

    # SM[b]: mtil2 lhsT; rhs = h1(b-2) tile (rows 0:2 of rhs are junk,
    # killed by zero rows here):  SM[b][2+t'', 2+t] = mtil2T
    SM = np.zeros((nb, MP, MP), np.float32)
    cross2 = np.einsum("btc,bsc->bst", Xb[2:], Xb[:-2])   # [j, t'', t]
    SM[2:, 2:, 2:] = cross2 * ub[:-2, :, None]
    SM = SM.astype(np.float16)

    # SW[b]: [2, 128] stale lhsT (X_b^T at cols 2:), rhs = w16(b-2) [2, UH]
    SW = np.zeros((nb, 2, MP), np.float32)
    SW[:, :, 2:] = Xb.transpose(0, 2, 1)
    SW = SW.astype(np.float16)

    # flatten to [part, nb*MP] streams
    kT_s = np.ascontiguousarray(kT.transpose(1, 0, 2).reshape(MP, nb * MP))
    CM_s = np.ascontiguousarray(CM.transpose(1, 0, 2).reshape(MP, nb * MP))
    SM_s = np.ascontiguousarray(SM.transpose(1, 0, 2).reshape(MP, nb * MP))
    SW_s = np.ascontiguousarray(SW.transpose(1, 0, 2).reshape(2, nb * MP))
    return kT_s, CM_s, SM_s, SW_s


# ---------------------------------------------------------------------------
# Device program
# ---------------------------------------------------------------------------

def build_nc(nb: int):
    nc = bacc.Bacc("TRN2", target_bir_lowering=False, debug=False)
    nch = (nb + CH - 1) // CH
    cols = nch * CH * MP
    kT_d = nc.declare_dram_parameter("kT", [MP, cols], F16, isOutput=False)
    CM_d = nc.declare_dram_parameter("CM", [MP, cols], F16, isOutput=False)
    SM_d = nc.declare_dram_parameter("SM", [MP, cols], F16, isOutput=False)
    SW_d = nc.declare_dram_parameter("SW", [2, cols], F16, isOutput=False)
    w0_d = nc.declare_dram_parameter("w0T", [2, UH], F32, isOutput=False)
    h2o_d = nc.declare_dram_parameter("h2o", [MP, nb * UH], F16,
                                      isOutput=True)

    with tile.TileContext(nc) as tc:
        _emit(tc, nc, nb, nch, kT_d, CM_d, SM_d, SW_d, w0_d, h2o_d)
    nc.compile()
    return nc


def _emit(tc, nc, nb, nch, kT_d, CM_d, SM_d, SW_d, w0_d, h2o_d):
    from contextlib import ExitStack
    with ExitStack() as ctx:
        singles = ctx.enter_context(tc.tile_pool(name="singles", bufs=1))
        pool_kt = ctx.enter_context(tc.tile_pool(name="ktc", bufs=3))
        pool_cm = ctx.enter_context(tc.tile_pool(name="cmc", bufs=3))
        pool_sm = ctx.enter_context(tc.tile_pool(name="smc", bufs=3))
        pool_sw = ctx.enter_context(tc.tile_pool(name="swc", bufs=3))
        pool_r = ctx.enter_context(tc.tile_pool(name="rbuf", bufs=6))
        pool_h0 = ctx.enter_context(tc.tile_pool(name="h0buf", bufs=4))
        pool_h0n = ctx.enter_context(tc.tile_pool(name="h0nbuf", bufs=4))
        pool_h2 = ctx.enter_context(tc.tile_pool(name="h2buf", bufs=8))
        pool_d21 = ctx.enter_context(tc.tile_pool(name="d21buf", bufs=4))
        pool_wh = ctx.enter_context(tc.tile_pool(name="whbuf", bufs=4))
        pool_w16 = ctx.enter_context(tc.tile_pool(name="w16buf", bufs=4))
        psum_a = ctx.enter_context(tc.tile_pool(name="psa", bufs=6,
                                                space="PSUM"))

        w0_sb = singles.tile([2, UH], F32)
        nc.sync.dma_start(out=w0_sb, in_=w0_d[:, :])

        kt_ch, cm_ch, sm_ch, sw_ch = {}, {}, {}, {}

        def load_chunk(j):
            if j >= nch:
                return
            s = j * CH * MP
            e = (j + 1) * CH * MP
            kt = pool_kt.tile([MP, CH * MP], F16, tag="kt")
            nc.sync.dma_start(out=kt, in_=kT_d[:, s:e])
            cm = pool_cm.tile([MP, CH * MP], F16, tag="cm")
            nc.sync.dma_start(out=cm, in_=CM_d[:, s:e])
            sm = pool_sm.tile([MP, CH * MP], F16, tag="sm")
            nc.sync.dma_start(out=sm, in_=SM_d[:, s:e])
            sw = pool_sw.tile([2, CH * MP], F16, tag="sw")
            nc.sync.dma_start(out=sw, in_=SW_d[:, s:e])
            kt_ch[j], cm_ch[j], sm_ch[j], sw_ch[j] = kt, cm, sm, sw

        def st_ap(store, b):
            o = (b % CH) * MP
            return store[b // CH][:, o:o + MP]

        load_chunk(0)
        load_chunk(1)

        # initial R (zero) / w16 (fp16 W0) tiles and the f32 W master
        r_init0 = pool_r.tile([MP, UH], F16, tag="r")
        nc.vector.memset(r_init0, 0.0)
        r_init1 = pool_r.tile([MP, UH], F16, tag="r")
        nc.vector.memset(r_init1, 0.0)
        w16_init0 = pool_w16.tile([2, UH], F16, tag="w16")
        nc.vector.tensor_copy(w16_init0, w0_sb)
        w16_init1 = pool_w16.tile([2, UH], F16, tag="w16")
        nc.vector.tensor_copy(w16_init1, w0_sb)
        wh_init = pool_wh.tile([2, UH], F32, tag="wh")
        nc.vector.tensor_copy(wh_init, w0_sb)

        A, R, H0, H0N, H2, WH, D21, W16 = {}, {}, {}, {}, {}, {}, {}, {}
        R[-2], R[-1] = r_init0, r_init1
        W16[-2], W16[-1] = w16_init0, w16_init1
        WH[-1] = wh_init

        def mm_sm(b):
            # big early part: mtil2 @ h1(b-2), group start
            a = psum_a.tile([MP, UH], F32, tag="a")
            A[b] = a
            nc.tensor.matmul(a, lhsT=st_ap(sm_ch, b), rhs=R[b - 2],
                             start=True, stop=False, skip_group_check=True)

        def mm_sw(b):
            # tiny stale part: X_b @ W(b-2) via K=2 matmul
            nc.tensor.matmul(A[b], lhsT=st_ap(sw_ch, b), rhs=W16[b - 2],
                             start=False, stop=False, skip_group_check=True)

        def sig0(b):
            h0 = pool_h0.tile([MP, UH], F16, tag="h0")
            H0[b] = h0
            nc.scalar.activation(h0, A[b], AF.Sigmoid)

        def neg_h0(b):
            h0n = pool_h0n.tile([MP, UH], F16, tag="h0n")
            H0N[b] = h0n
            nc.vector.tensor_scalar(h0n, H0[b], -1.0, None, OP.mult)

        def mm_kh0(b):
            nc.tensor.matmul(A[b], lhsT=st_ap(kt_ch, b), rhs=H0[b],
                             start=False, stop=False, skip_group_check=True)

        def mm_comb1(b):
            nc.tensor.matmul(A[b], lhsT=st_ap(cm_ch, b), rhs=R[b - 1],
                             start=False, stop=False, skip_group_check=True)

        def sig1(b):
            r = pool_r.tile([MP, UH], F16, tag="r")
            R[b] = r
            nc.scalar.activation(r, A[b], AF.Sigmoid)

        def mm_knh0(b):
            nc.tensor.matmul(A[b], lhsT=st_ap(kt_ch, b), rhs=H0N[b],
                             start=False, stop=False, skip_group_check=True)

        def mm_kh1(b, stop=False):
            nc.tensor.matmul(A[b], lhsT=st_ap(kt_ch, b), rhs=R[b],
                             start=False, stop=stop, skip_group_check=True)

        def mm_comb2d(b):
            nc.tensor.matmul(A[b], lhsT=st_ap(cm_ch, b), rhs=D21[b - 1],
                             start=False, stop=True, skip_group_check=True)

        def sig2(b):
            h2 = pool_h2.tile([MP, UH], F16, tag="h2")
            H2[b] = h2
            nc.scalar.activation(h2, A[b], AF.Sigmoid)

        def dma_h2(b):
            nc.sync.dma_start(out=h2o_d[:, b * UH:(b + 1) * UH], in_=H2[b])

        def dve_d21(b):
            d = pool_d21.tile([MP, UH], F16, tag="d21")
            D21[b] = d
            nc.vector.tensor_tensor(d, H2[b], R[b], OP.subtract)

        def dve_whadd(b):
            wh = pool_wh.tile([2, UH], F32, tag="wh")
            nc.vector.tensor_tensor(wh, WH[b - 1], A[b][0:2, :], OP.add)
            WH[b] = wh

        def dve_whcopy(b):
            w = pool_w16.tile([2, UH], F16, tag="w16")
            W16[b] = w
            nc.vector.tensor_tensor(w, WH[b - 1], A[b][0:2, :], OP.add)

        # ---- prologue: blocks 0 and 1 seeded ----
        mm_sm(0)
        mm_sw(0)
        mm_sm(1)
        mm_sw(1)
        sig0(0)
        neg_h0(0)
        mm_kh0(0)
        sig1(0)

        EST = 0.00025
        for b in range(nb):
            tc.tile_set_cur_wait(b * EST)
            if b % CH == 0:
                load_chunk(b // CH + 2)
            if b >= 1:
                dve_d21(b - 1)
            mm_knh0(b)
            mm_kh1(b, stop=(b == 0))
            if b >= 1:
                mm_comb2d(b)
            if b + 1 < nb:
                sig0(b + 1)
                mm_comb1(b + 1)
                mm_kh0(b + 1)
            sig2(b)
            dma_h2(b)
            dve_whadd(b)
            if b + 1 < nb:
                neg_h0(b + 1)
                dve_whcopy(b)
            if b + 2 < nb:
                mm_sm(b + 2)
                mm_sw(b + 2)
            if b + 1 < nb:
                sig1(b + 1)
            for d, store in ((4, A), (6, R), (3, H0), (3, H0N), (7, H2),
                             (3, D21), (3, WH), (3, W16)):
                store.pop(b - d, None)


# ---------------------------------------------------------------------------
# Host wrapper
# ---------------------------------------------------------------------------

def run_cores(X, rewards, W0, nb, t_real, trace=False):
    kT_s, CM_s, SM_s, SW_s = prep_streams(X, rewards, nb)
    nch = (nb + CH - 1) // CH
    cols = nch * CH * MP

    def pad(a):
        if a.shape[1] < cols:
            b = np.zeros((a.shape[0], cols), a.dtype)
            b[:, :a.shape[1]] = a
            return b
        return a

    kT_s, CM_s, SM_s, SW_s = pad(kT_s), pad(CM_s), pad(SM_s), pad(SW_s)

    nc = build_nc(nb)
    in_maps = []
    for c in range(NCORES):
        w0c = np.ascontiguousarray(W0[c * UH:(c + 1) * UH].T)  # [2, UH] f32
        in_maps.append({"kT": kT_s, "CM": CM_s, "SM": SM_s, "SW": SW_s,
                        "w0T": w0c.astype(np.float32)})
    res = run_bass_kernel_spmd(nc, in_maps, list(range(NCORES)), trace=trace)
    total = np.zeros((TAU, nb), np.float64)
    for c in range(NCORES):
        h2o = res.results[c]["h2o"]                  # [MP, nb*UH] f16
        total += h2o[2:MP].reshape(TAU, nb, UH).astype(np.float64).sum(axis=2)
    m = (total / float(NH)).T.reshape(-1)[:t_real].astype(np.float32)
    return m, res


def kernel(X, rewards, W_plastic_init):
    m, _ = run_cores(np.asarray(X, np.float32),
                     np.asarray(rewards, np.float32),
                     np.asarray(W_plastic_init, np.float32),
                     NB_FULL, T_FULL)
    return m



# revision 21
# speedup vs baseline: 1.6370x; 1.6370x over previous
"""Trainium2 Bass kernel for nn_BehavioralCircuit — v3 superblock solver.

Reference: T=100000 sequential steps of a reward-modulated Hebbian rule over
512 independent 2-D units:
    r[t] = rewards[t] - movavg10(rewards)[t];  u = LR*r
    h    = sigmoid(W @ x_t);  m[t] = h.mean();  W += u[t] * outer(h, x_t)

Scheme (validated against a numpy mirror; full-run rel err ~6e-3):
  Superblocks of SB=252 steps = 2 tiles (a,b) of TAU=126.  Per sb s,
  one PSUM tile A[s] [128, 128] holds both tiles' pre-activations
  (64 units per column half).  Two wide sigmoids per sb:
    h1(s) = sig(X W(s-2)_ledger + X.(c^T h1)(s-2) + CR(s-1->s) h1(s-1)
               + 0.5-seeded within-coupling bias)        [one 128-wide ACT]
    h2(s) = sig(same + K_within (h1(s) - 0.5))           [one 128-wide ACT]
  The 0.5 seed is folded into a host-precomputed bias row of the tiny
  base matmul; its removal at the h2 stage is folded into row 126 of the
  within matrices (h tiles' row 126 is exactly sig(0)=0.5).
  Cross couplings older than one sb are rank-2 (diag(u) X_src X_tgt^T) and
  fold into an f32 weight ledger via tiny c^T h matmuls; only the 3 within
  mats (fp16) and 4 prev-sb cross mats (fp8e4) are streamed from HBM.
  The only tight serial chain is sig1(s-1) -> 4 CR matmuls -> sig1(s).

Per core: 64 units; 8 cores partition the 512 units; host sums unit means.
"""

import sys

import numpy as np

sys.path.insert(0, "/opt/trn_rl_repo")

import concourse.bass as bass
import concourse.bacc as bacc
import concourse.tile as tile
from concourse import mybir
from concourse.bass_utils import run_bass_kernel_spmd

TAU = 126
MP = 128
G = 2
SB = G * TAU            # 252
T_FULL = 100000
NSB_FULL = (T_FULL + SB - 1) // SB   # 397
NB_FULL = NSB_FULL      # test.py compat (loop count = superblocks)
NH = 512
NCORES = 8
UH = NH // NCORES       # 64
CH = 16                 # superblocks per DMA chunk
LR = 0.1
WINDOW = 10

F32 = mybir.dt.float32
F16 = mybir.dt.float16
F8 = mybir.dt.float8e4
AF = mybir.ActivationFunctionType
OP = mybir.AluOpType


# ---------------------------------------------------------------------------
# Host-side stream preparation (shared across cores)
# ---------------------------------------------------------------------------

def _movavg_u(rewards, t_pad):
    cs = np.cumsum(rewards, dtype=np.float64)
    sh = np.concatenate([np.zeros(WINDOW), cs[:-WINDOW]])
    wsum = cs - sh
    count = np.minimum(np.arange(len(rewards)) + 1.0, float(WINDOW))
    u = (LR * (rewards - wsum / count)).astype(np.float32)
    up = np.zeros((t_pad,), np.float32)
    up[: len(rewards)] = u
    return up


def prep_streams(X, rewards, nsb):
    t_pad = nsb * SB
    Xp = np.zeros((t_pad, 2), np.float32)
    Xp[: X.shape[0]] = X
    up = _movavg_u(rewards, t_pad)
    Xa = Xp.reshape(nsb, SB, 2)[:, :TAU]          # [s, 126, 2]
    Xb = Xp.reshape(nsb, SB, 2)[:, TAU:]
    ua = up.reshape(nsb, SB)[:, :TAU]
    ub = up.reshape(nsb, SB)[:, TAU:]

    strict = np.triu(np.ones((TAU, TAU), np.float32), 1)   # [sp, t]: sp < t

    def cmat(Xs, us, Xt, mask):
        M = np.einsum("spc,stc->spt", Xs, Xt) * us[:, :, None]
        if mask is not None:
            M = M * mask[None]
        return M

    KAA = cmat(Xa, ua, Xa, strict)
    KAB = cmat(Xa, ua, Xb, None)
    KBB = cmat(Xb, ub, Xb, strict)
    bias_a = 0.5 * KAA.sum(axis=1)                 # [s, 126]
    bias_b = 0.5 * (KAB.sum(axis=1) + KBB.sum(axis=1))

    # within stream fp16 [128, nsb*384]: per sb: KAA | KAB | KBB
    WM = np.zeros((nsb, 3, MP, MP), np.float32)
    WM[:, 0, :TAU, :TAU] = KAA
    WM[:, 1, :TAU, :TAU] = KAB
    WM[:, 2, :TAU, :TAU] = KBB
    WM[:, 0, TAU, :TAU] = -2.0 * bias_a            # * h1row126 (=0.5)
    WM[:, 2, TAU, :TAU] = -2.0 * bias_b
    WM = WM.transpose(2, 0, 1, 3).reshape(MP, nsb * 3 * MP).astype(np.float16)

    # cross stream fp8e4 [128, nsb*512]: per sb s (sources sb s-1):
    # CR[a'->a] | CR[a'->b] | CR[b'->a] | CR[b'->b]   (zero for s=0)
    CRf = np.zeros((nsb, 4, MP, MP), np.float32)
    CRf[1:, 0, :TAU, :TAU] = cmat(Xa[:-1], ua[:-1], Xa[1:], None)
    CRf[1:, 1, :TAU, :TAU] = cmat(Xa[:-1], ua[:-1], Xb[1:], None)
    CRf[1:, 2, :TAU, :TAU] = cmat(Xb[:-1], ub[:-1], Xa[1:], None)
    CRf[1:, 3, :TAU, :TAU] = cmat(Xb[:-1], ub[:-1], Xb[1:], None)
    CR = CRf.transpose(2, 0, 1, 3).reshape(MP, nsb * 4 * MP)
    CR = CR.astype(mybir.dt.np(F8))

    # SW lhsT fp16 [3, nsb*256]: per sb: SW_a | SW_b
    # rows 0:2 = X^T (contracts [W; ones] and, sliced [0:2], the D1 tile),
    # row 2 = +bias (contracts the ones row)
    SW = np.zeros((nsb, 2, 3, MP), np.float32)
    SW[:, 0, 0:2, :TAU] = Xa.transpose(0, 2, 1)
    SW[:, 1, 0:2, :TAU] = Xb.transpose(0, 2, 1)
    SW[:, 0, 2, :TAU] = bias_a
    SW[:, 1, 2, :TAU] = bias_b
    SW = SW.transpose(2, 0, 1, 3).reshape(3, nsb * 2 * MP).astype(np.float16)

    # C stream fp16 [128, nsb*6]: per sb: c_a (3 cols) | c_b (3 cols),
    # third col zero so the D psum [3, 64] keeps row 2 == 0 (ones-row guard)
    C = np.zeros((nsb, MP, 6), np.float32)
    C[:, :TAU, 0:2] = ua[:, :, None] * Xa
    C[:, :TAU, 3:5] = ub[:, :, None] * Xb
    C = C.transpose(1, 0, 2).reshape(MP, nsb * 6).astype(np.float16)

    return WM, CR, SW, C


# ---------------------------------------------------------------------------
# Device program
# ---------------------------------------------------------------------------

def build_nc(nsb: int):
    nc = bacc.Bacc("TRN2", target_bir_lowering=False, debug=False)
    nch = (nsb + CH - 1) // CH
    WM_d = nc.declare_dram_parameter("WM", [MP, nch * CH * 3 * MP], F16,
                                     isOutput=False)
    CR_d = nc.declare_dram_parameter("CR", [MP, nch * CH * 4 * MP], F8,
                                     isOutput=False)
    SW_d = nc.declare_dram_parameter("SW", [3, nch * CH * 2 * MP], F16,
                                     isOutput=False)
    C_d = nc.declare_dram_parameter("C", [MP, nch * CH * 6], F16,
                                    isOutput=False)
    w0_d = nc.declare_dram_parameter("w0T", [3, UH], F32, isOutput=False)
    h2o_d = nc.declare_dram_parameter("h2o", [MP, nsb * MP], F16,
                                      isOutput=True)

    with tile.TileContext(nc) as tc:
        _emit(tc, nc, nsb, nch, WM_d, CR_d, SW_d, C_d, w0_d, h2o_d)
    nc.compile()
    return nc


def _emit(tc, nc, nsb, nch, WM_d, CR_d, SW_d, C_d, w0_d, h2o_d):
    from contextlib import ExitStack
    with ExitStack() as ctx:
        singles = ctx.enter_context(tc.tile_pool(name="singles", bufs=5))
        pool_wm = ctx.enter_context(tc.tile_pool(name="wmc", bufs=3))
        pool_cr = ctx.enter_context(tc.tile_pool(name="crc", bufs=3))
        pool_sw = ctx.enter_context(tc.tile_pool(name="swc", bufs=3))
        pool_c = ctx.enter_context(tc.tile_pool(name="cc", bufs=3))
        pool_h1 = ctx.enter_context(tc.tile_pool(name="h1buf", bufs=3))
        pool_h2 = ctx.enter_context(tc.tile_pool(name="h2buf", bufs=3))
        pool_wh = ctx.enter_context(tc.tile_pool(name="whbuf", bufs=3))
        psum_a = ctx.enter_context(tc.tile_pool(name="psa", bufs=3,
                                                space="PSUM"))
        psum_d = ctx.enter_context(tc.tile_pool(name="psd", bufs=2,
                                                space="PSUM"))
        psum_d1 = ctx.enter_context(tc.tile_pool(name="psd1", bufs=2,
                                                 space="PSUM"))

        w0_sb = singles.tile([3, UH], F32)
        nc.sync.dma_start(out=w0_sb, in_=w0_d[:, :])

        # rhs3: persistent [3, UH] f16 tiles (rows 0:2 W, row 2 ones);
        # d1c: [2, UH] f16 tiles (D1 rank-2 correction)
        rhs3_0 = singles.tile([3, UH], F16, tag="rhs3_0")
        rhs3_1 = singles.tile([3, UH], F16, tag="rhs3_1")
        d1c_0 = singles.tile([2, UH], F16, tag="d1c_0")
        d1c_1 = singles.tile([2, UH], F16, tag="d1c_1")
        rhs3 = [rhs3_0, rhs3_1]
        d1c = [d1c_0, d1c_1]
        for r in rhs3:
            nc.vector.tensor_copy(r, w0_sb)
        for r in d1c:
            nc.vector.memset(r, 0.0)

        wh_init = pool_wh.tile([3, UH], F32, tag="wh")
        nc.vector.tensor_copy(wh_init, w0_sb)

        wm_ch, cr_ch, sw_ch, c_ch = {}, {}, {}, {}

        def load_chunk(j):
            if j >= nch:
                return
            wm = pool_wm.tile([MP, CH * 3 * MP], F16, tag="wm")
            nc.sync.dma_start(out=wm, in_=WM_d[:, j * CH * 3 * MP:
                                               (j + 1) * CH * 3 * MP])
            cr = pool_cr.tile([MP, CH * 4 * MP], F8, tag="cr")
            nc.sync.dma_start(out=cr, in_=CR_d[:, j * CH * 4 * MP:
                                               (j + 1) * CH * 4 * MP])
            sw = pool_sw.tile([3, CH * 2 * MP], F16, tag="sw")
            nc.sync.dma_start(out=sw, in_=SW_d[:, j * CH * 2 * MP:
                                               (j + 1) * CH * 2 * MP])
            cc = pool_c.tile([MP, CH * 6], F16, tag="c")
            nc.sync.dma_start(out=cc, in_=C_d[:, j * CH * 6:(j + 1) * CH * 6])
            wm_ch[j], cr_ch[j], sw_ch[j], c_ch[j] = wm, cr, sw, cc

        def wm_ap(s, k):      # k in 0..2: KAA, KAB, KBB
            o = (s % CH) * 3 * MP + k * MP
            return wm_ch[s // CH][:, o:o + MP]

        def cr_ap(s, k):      # k in 0..3
            o = (s % CH) * 4 * MP + k * MP
            return cr_ch[s // CH][:, o:o + MP]

        def sw_ap(s, k):      # k in 0..1
            o = (s % CH) * 2 * MP + k * MP
            return sw_ch[s // CH][:, o:o + MP]

        def c_ap(s, k, w):    # k in 0..1: c_a, c_b; w = 2 (D1) or 3 (D)
            o = (s % CH) * 6 + k * 3
            return c_ch[s // CH][:, o:o + w]

        load_chunk(0)
        load_chunk(1)

        A, H1, H2, D, D1, WH = {}, {}, {}, {}, {}, {}
        WH[-1] = wh_init

        HA = slice(0, UH)          # column half a
        HB = slice(UH, 2 * UH)

        def mm(out, lhsT, rhs, start, stop):
            nc.tensor.matmul(out, lhsT=lhsT, rhs=rhs, start=start, stop=stop,
                             skip_group_check=True)

        def emit_base(t):
            # A[t] base: CR(t-1 -> t) @ h1(t-1) + SW(t); t >= 1
            a = psum_a.tile([MP, 2 * UH], F32, tag="a")
            A[t] = a
            # exactly ONE start=True per psum tile: a second start on the
            # same tile wipes the first matmul's accumulation (observed on
            # HW).  The start resets the whole tile region.
            h1p = H1[t - 1]
            mm(a[:, HA], cr_ap(t, 0), h1p[:, HA], True, False)
            mm(a[:, HB], cr_ap(t, 1), h1p[:, HA], False, False)
            mm(a[:, HA], cr_ap(t, 2), h1p[:, HB], False, False)
            mm(a[:, HB], cr_ap(t, 3), h1p[:, HB], False, False)
            mm(a[:, HA], sw_ap(t, 0), rhs3[t % 2], False, False)
            mm(a[:, HB], sw_ap(t, 1), rhs3[t % 2], False, False)
            mm(a[:, HA], sw_ap(t, 0)[0:2, :], d1c[t % 2], False, False)
            mm(a[:, HB], sw_ap(t, 1)[0:2, :], d1c[t % 2], False, False)

        def emit_within(s):
            a = A[s]
            h1 = H1[s]
            mm(a[:, HA], wm_ap(s, 0), h1[:, HA], False, True)   # KAA, stop a
            mm(a[:, HB], wm_ap(s, 1), h1[:, HA], False, False)  # KAB
            mm(a[:, HB], wm_ap(s, 2), h1[:, HB], False, True)   # KBB, stop b

        def emit_d1(s):
            d1 = psum_d1.tile([2, UH], F32, tag="d1")
            D1[s] = d1
            mm(d1, c_ap(s, 0, 2), H1[s][:, HA], True, False)
            mm(d1, c_ap(s, 1, 2), H1[s][:, HB], False, True)

        def emit_d(s):
            d = psum_d.tile([3, UH], F32, tag="d")
            D[s] = d
            mm(d, c_ap(s, 0, 3), H2[s][:, HA], True, False)
            mm(d, c_ap(s, 1, 3), H2[s][:, HB], False, True)

        def sig1(s):
            h1 = pool_h1.tile([MP, 2 * UH], F16, tag="h1")
            H1[s] = h1
            nc.scalar.activation(h1, A[s], AF.Sigmoid)

        def sig2(s):
            h2 = pool_h2.tile([MP, 2 * UH], F16, tag="h2")
            H2[s] = h2
            nc.scalar.activation(h2, A[s], AF.Sigmoid)

        def dma_h2(s):
            nc.sync.dma_start(out=h2o_d[:, s * MP:(s + 1) * MP], in_=H2[s])

        # ---- prologue: A[0] = SW(0) only.  Each psum half-group must be
        # STARTED by a 128-K matmul (small-K start + big-K accumulate breaks
        # the psum group); the s=0 CR mats are all-zero, so use them with a
        # junk rhs as the group opener.
        a0 = psum_a.tile([MP, 2 * UH], F32, tag="a")
        A[0] = a0
        junk = wm_ch[0][:, 0:UH]
        mm(a0[:, HA], cr_ap(0, 0), junk, True, False)
        mm(a0[:, HB], cr_ap(0, 1), junk, False, False)
        mm(a0[:, HA], sw_ap(0, 0), rhs3[0], False, False)
        mm(a0[:, HB], sw_ap(0, 1), rhs3[0], False, False)

        EST = 0.00045
        for s in range(nsb):
            tc.tile_set_cur_wait(s * EST)
            if s % CH == 0:
                load_chunk(s // CH + 2)
            # ACT
            sig1(s)
            if s >= 1:
                sig2(s - 1)
                dma_h2(s - 1)
            # PE
            if s + 1 < nsb:
                emit_base(s + 1)
            emit_within(s)
            emit_d1(s)
            if s >= 1:
                emit_d(s - 1)
            # DVE ledger: rhs3/d1c[s % 2] serve SW(s+2)
            if s + 2 < nsb:
                nc.vector.tensor_copy(d1c[s % 2], D1[s])
            if s >= 1:
                wh = pool_wh.tile([3, UH], F32, tag="wh")
                nc.vector.tensor_tensor(wh, WH[s - 1], D[s - 1], OP.add)
                WH[s] = wh
                if s + 2 < nsb:
                    nc.vector.tensor_copy(rhs3[s % 2], wh)
            else:
                WH[0] = WH[-1]
            for dlag, store in ((3, A), (3, H1), (3, H2), (2, D), (2, D1),
                                (3, WH)):
                store.pop(s - dlag, None)

        # epilogue
        sig2(nsb - 1)
        dma_h2(nsb - 1)


# ---------------------------------------------------------------------------
# Host wrapper
# ---------------------------------------------------------------------------

def run_cores(X, rewards, W0, nsb, t_real, trace=False):
    WM, CR, SW, C = prep_streams(X, rewards, nsb)
    nch = (nsb + CH - 1) // CH
    cols = {"WM": nch * CH * 3 * MP, "CR": nch * CH * 4 * MP,
            "SW": nch * CH * 2 * MP, "C": nch * CH * 6}

    def pad(a, c):
        if a.shape[1] < c:
            b = np.zeros((a.shape[0], c), a.dtype)
            b[:, :a.shape[1]] = a
            return b
        return a

    WM, CR = pad(WM, cols["WM"]), pad(CR, cols["CR"])
    SW, C = pad(SW, cols["SW"]), pad(C, cols["C"])

    nc = build_nc(nsb)
    in_maps = []
    for c in range(NCORES):
        w0c = np.zeros((3, UH), np.float32)
        w0c[0:2] = W0[c * UH:(c + 1) * UH].T
        w0c[2] = 1.0
        in_maps.append({"WM": WM, "CR": CR, "SW": SW, "C": C, "w0T": w0c})
    res = run_bass_kernel_spmd(nc, in_maps, list(range(NCORES)), trace=trace)
    # h2o rows 0:126 are steps; rows 126:128 are sig(0)=0.5 junk
    total = np.zeros((nsb, SB), np.float64)
    for c in range(NCORES):
        h2o = res.results[c]["h2o"].astype(np.float64)
        per = h2o.reshape(MP, nsb, 2, UH)            # [row, s, tile, unit]
        su = per[:TAU].sum(axis=3)                   # [126, s, 2]
        total += su.transpose(1, 2, 0).reshape(nsb, SB)
    m = (total / float(NH)).reshape(-1)[:t_real].astype(np.float32)
    return m, res


def kernel(X, rewards, W_plastic_init):
    m, _ = run_cores(np.asarray(X, np.float32),
                     np.asarray(rewards, np.float32),
                     np.asarray(W_plastic_init, np.float32),
                     NSB_FULL, T_FULL)
    return m
